# revision 1
# baseline (speedup 1.0000x reference)
"""Distributed Trainium2 Bass kernel for a 4-layer GPT-style transformer.

Sharding: 8 cores = 2 batch groups x 4 vocab shards.
  - core c: batch element g = c//4, vocab shard j = c%4 (12672 ids, padded).
  - Transformer body computed per batch element (replicated within each
    group of 4); tied LM head sharded over vocab.  No collectives.

On-chip layout: activations transposed (features on partitions, tokens on
free).  LayerNorm stats via ones-matmul partition reductions; attention via
transposed scores (k @ q^T) so probabilities land keys-on-partitions, ready
for the A@V matmul with no transposes.  Softmax skips max-subtraction
(|scores| < ~2 by construction); causality = 0/1 mask multiply after exp,
only on diagonal-crossing tiles.  Matmuls bf16, residual stream fp32.
Big weight matrices are streamed from DRAM per output tile.
"""

import numpy as np
import ml_dtypes

import concourse.bass as bass
import concourse.mybir as mybir
import concourse.tile as tile
from concourse import bacc
from concourse.bass_utils import run_bass_kernel_spmd

V, E, NH, HD, L, T, B, FF = 50257, 768, 12, 64, 4, 1024, 2, 3072
EPS = 1e-5
P = 128
KE = E // P            # 6 feature subtiles
KF = FF // P           # 24
NT = T // P            # 8 token tiles
NC = 512               # matmul free-dim chunk
NCH = T // NC          # 2 chunks
VP = 12672             # vocab shard per core (99 * 128)
MV = VP // P           # 99
BF16 = mybir.dt.bfloat16
F32 = mybir.dt.float32
AF = mybir.ActivationFunctionType
OP = mybir.AluOpType
BF = ml_dtypes.bfloat16

_CACHE = {}


def _build():
    nc = bacc.Bacc("TRN2", target_bir_lowering=False, debug=False,
                   num_devices=8)

    x0t = nc.declare_dram_parameter("x0t", [E, T], F32, isOutput=False)
    wqk = nc.declare_dram_parameter("wqk", [L, E, 2 * E], BF16, isOutput=False)
    wv = nc.declare_dram_parameter("wv", [L, E, E], BF16, isOutput=False)
    wout = nc.declare_dram_parameter("wout", [L, E, E], BF16, isOutput=False)
    wfc1 = nc.declare_dram_parameter("wfc1", [L, E, FF], BF16, isOutput=False)
    bfc1 = nc.declare_dram_parameter("bfc1", [L, P, KF], F32, isOutput=False)
    wfc2 = nc.declare_dram_parameter("wfc2", [L, FF, E], BF16, isOutput=False)
    bfc2 = nc.declare_dram_parameter("bfc2", [L, P, KE], F32, isOutput=False)
    wemb = nc.declare_dram_parameter("wemb", [E, VP], BF16, isOutput=False)
    maskp = nc.declare_dram_parameter("mask", [4, P, NC], BF16, isOutput=False)
    out = nc.declare_dram_parameter("out", [VP, T], F32, isOutput=True)

    with tile.TileContext(nc) as tc:
        with (
            tc.tile_pool(name="resident", bufs=1) as res,
            tc.tile_pool(name="wts", bufs=1) as wpool,
            tc.tile_pool(name="acts", bufs=1) as apool,
            tc.tile_pool(name="wstream", bufs=3) as wst,
            tc.tile_pool(name="small", bufs=3) as spool,
            tc.tile_pool(name="small2", bufs=2) as spool2,
            tc.tile_pool(name="ps", bufs=2, space="PSUM") as psp,
        ):
            # --- resident tiles ---
            x = res.tile([P, KE, T], F32)          # residual stream (xT)
            xhat = res.tile([P, KE, T], BF16)      # normalized, bf16
            mask = res.tile([P, 4, NC], BF16)      # diagonal masks
            ones_c = res.tile([P, 1], BF16)
            ones_r = res.tile([1, P], F32)
            negmb = res.tile([P, T], F32)          # -mean broadcast
            rstdb = res.tile([P, T], F32)          # rstd broadcast
            stat = res.tile([1, 2, T], F32)        # negmean / rstd rows
            eps_c = res.tile([1, 1], F32)

            nc.any.memset(ones_c[:], 1.0)
            nc.any.memset(ones_r[:], 1.0)
            nc.any.memset(eps_c[:], EPS)
            nc.sync.dma_start(mask[:], maskp.ap().rearrange("n p t -> p n t"))
            nc.sync.dma_start(x[:], x0t.ap().rearrange("(ko p) t -> p ko t",
                                                       p=P))

            def layernorm():
                """x (f32) -> xhat (bf16), pure normalize (scales folded)."""
                for c in range(NCH):
                    cs = slice(c * NC, (c + 1) * NC)
                    ps_s = psp.tile([1, NC], F32, tag="st")
                    ps_q = psp.tile([1, NC], F32, tag="st")
                    xbts = []
                    for k in range(KE):
                        xbt = spool.tile([P, NC], BF16, tag="xbt")
                        nc.vector.tensor_copy(out=xbt[:], in_=x[:, k, cs])
                        nc.tensor.matmul(ps_s, ones_c[:], xbt[:],
                                         start=(k == 0), stop=(k == KE - 1))
                        xbts.append(xbt)
                    for k in range(KE):
                        xsq = spool.tile([P, NC], BF16, tag="xsq")
                        nc.vector.tensor_tensor(
                            xsq[:], xbts[k][:], xbts[k][:], OP.mult)
                        nc.tensor.matmul(ps_q, ones_c[:], xsq[:],
                                         start=(k == 0), stop=(k == KE - 1))
                    t_m = spool2.tile([1, NC], F32, tag="t_m")
                    t_v = spool2.tile([1, NC], F32, tag="t_v")
                    nc.vector.tensor_scalar_mul(stat[:, 0, cs], ps_s,
                                                -1.0 / E)
                    nc.vector.tensor_scalar_mul(t_m, ps_s, 1.0 / E)
                    nc.vector.tensor_scalar_mul(t_v, ps_q, 1.0 / E)
                    nc.vector.tensor_tensor(t_m, t_m, t_m, OP.mult)
                    nc.vector.tensor_tensor(t_v, t_v, t_m, OP.subtract)
                    nc.scalar.activation(t_v, t_v, AF.Sqrt, bias=eps_c[:])
                    nc.vector.reciprocal(stat[:, 1, cs], t_v)
                    ps_b = psp.tile([P, NC], F32, tag="bc")
                    nc.tensor.matmul(ps_b, ones_r[:], stat[:, 0, cs],
                                     start=True, stop=True)
                    nc.vector.tensor_copy(out=negmb[:, cs], in_=ps_b)
                    ps_b2 = psp.tile([P, NC], F32, tag="bc")
                    nc.tensor.matmul(ps_b2, ones_r[:], stat[:, 1, cs],
                                     start=True, stop=True)
                    nc.vector.tensor_copy(out=rstdb[:, cs], in_=ps_b2)
                    for k in range(KE):
                        tmp = spool2.tile([P, NC], F32, tag="lntmp")
                        nc.vector.tensor_tensor(
                            tmp, x[:, k, cs], negmb[:, cs], OP.add)
                        nc.vector.tensor_tensor(
                            xhat[:, k, cs], tmp, rstdb[:, cs], OP.mult)

            def w6(dram_ap, m):
                """Stream a (128, KE, 128) lhsT block for output tile m."""
                wt = wst.tile([P, KE, P], BF16, tag="wm6")
                nc.sync.dma_start(
                    wt[:], dram_ap[:, m * P:(m + 1) * P].rearrange(
                        "(ko p) f -> p ko f", p=P))
                return wt

            for l in range(L):
                wv_s = wpool.tile([P, KE, E], BF16, tag="wv")
                b1_s = wpool.tile([P, KF], F32, tag="b1")
                b2_s = wpool.tile([P, KE], F32, tag="b2")
                nc.sync.dma_start(
                    wv_s[:], wv.ap()[l].rearrange("(ko p) f -> p ko f", p=P))
                nc.sync.dma_start(b1_s[:], bfc1.ap()[l])
                nc.sync.dma_start(b2_s[:], bfc2.ap()[l])

                layernorm()

                # ---- QK projection: qkT (2E, T) ----
                qk_t = apool.tile([P, 2 * KE, T], BF16, tag="qkt")
                for m in range(2 * KE):
                    wt = w6(wqk.ap()[l], m)
                    for c in range(NCH):
                        cs = slice(c * NC, (c + 1) * NC)
                        ps = psp.tile([P, NC], F32, tag="mm")
                        for k in range(KE):
                            nc.tensor.matmul(
                                ps, wt[:, k, :], xhat[:, k, cs],
                                start=(k == 0), stop=(k == KE - 1))
                        nc.vector.tensor_copy(out=qk_t[:, m, cs], in_=ps)

                # ---- V projection in (T, E) layout ----
                v_s = apool.tile([P, NT, E], BF16, tag="vs")
                for t in range(NT):
                    for (f0, fn) in ((0, NC), (NC, E - NC)):
                        ps = psp.tile([P, NC], F32, tag="mm")
                        for k in range(KE):
                            nc.tensor.matmul(
                                ps[:, :fn], xhat[:, k, t * P:(t + 1) * P],
                                wv_s[:, k, f0:f0 + fn],
                                start=(k == 0), stop=(k == KE - 1))
                        nc.vector.tensor_copy(
                            out=v_s[:, t, f0:f0 + fn], in_=ps[:, :fn])

                # ---- attention per head ----
                o_t = apool.tile([P, KE, T], BF16, tag="ot")
                for h in range(NH):
                    mt, mo = divmod(h * HD, P)
                    q_sl = qk_t[mo:mo + HD, mt, :]
                    k_sl = qk_t[mo:mo + HD, KE + mt, :]
                    for c in range(NCH):
                        cs = slice(c * NC, (c + 1) * NC)
                        ntk = 4 * (c + 1)   # causal: keep tk tiles 0..ntk-1
                        pts = []
                        for tk in range(ntk):
                            ps_s = psp.tile([P, NC], F32, tag="mm")
                            nc.tensor.matmul(
                                ps_s, k_sl[:, tk * P:(tk + 1) * P],
                                q_sl[:, cs], start=True, stop=True)
                            pt = spool.tile([P, NC], BF16, tag="pt")
                            nc.scalar.activation(pt, ps_s, AF.Exp)
                            d = tk - 4 * c
                            if d >= 0:   # diagonal-crossing tile: mask
                                nc.vector.tensor_tensor(
                                    pt, pt, mask[:, d, :], OP.mult)
                            pts.append(pt)
                        ps_o = psp.tile([P, NC], F32, tag="av")
                        ps_n = psp.tile([1, NC], F32, tag="st")
                        for i, pt in enumerate(pts):
                            nc.tensor.matmul(
                                ps_o[:HD], v_s[:, i, h * HD:(h + 1) * HD], pt,
                                start=(i == 0), stop=(i == ntk - 1))
                            nc.tensor.matmul(
                                ps_n, ones_c[:], pt,
                                start=(i == 0), stop=(i == ntk - 1))
                        rin = spool.tile([1, NC], F32, tag="rin")
                        nc.vector.reciprocal(rin, ps_n)
                        ps_r = psp.tile([P, NC], F32, tag="bc")
                        nc.tensor.matmul(ps_r[:HD], ones_r[:, :HD], rin,
                                         start=True, stop=True)
                        rb = spool.tile([P, NC], F32, tag="rb")
                        nc.vector.tensor_copy(out=rb[:HD], in_=ps_r[:HD])
                        nc.vector.tensor_tensor(
                            o_t[mo:mo + HD, mt, cs], ps_o[:HD], rb[:HD],
                            OP.mult)

                # ---- output projection + residual ----
                for m in range(KE):
                    wt = w6(wout.ap()[l], m)
                    for c in range(NCH):
                        cs = slice(c * NC, (c + 1) * NC)
                        ps = psp.tile([P, NC], F32, tag="mm")
                        for k in range(KE):
                            nc.tensor.matmul(
                                ps, wt[:, k, :], o_t[:, k, cs],
                                start=(k == 0), stop=(k == KE - 1))
                        nc.vector.tensor_tensor(
                            x[:, m, cs], ps, x[:, m, cs], OP.add)

                layernorm()

                # ---- FFN, one 512-token chunk at a time ----
                for c in range(NCH):
                    cs = slice(c * NC, (c + 1) * NC)
                    h1c = apool.tile([P, KF, NC], BF16, tag="h1c")
                    for m in range(KF):
                        wt = w6(wfc1.ap()[l], m)
                        ps = psp.tile([P, NC], F32, tag="mm")
                        for k in range(KE):
                            nc.tensor.matmul(
                                ps, wt[:, k, :], xhat[:, k, cs],
                                start=(k == 0), stop=(k == KE - 1))
                        nc.scalar.activation(
                            h1c[:, m, :], ps, AF.Gelu, bias=b1_s[:, m:m + 1])
                    for m in range(KE):
                        wt24 = wst.tile([P, KF, P], BF16, tag="wm24")
                        nc.sync.dma_start(
                            wt24[:],
                            wfc2.ap()[l][:, m * P:(m + 1) * P].rearrange(
                                "(ko p) f -> p ko f", p=P))
                        ps = psp.tile([P, NC], F32, tag="mm")
                        for k in range(KF):
                            nc.tensor.matmul(
                                ps, wt24[:, k, :], h1c[:, k, :],
                                start=(k == 0), stop=(k == KF - 1))
                        tmp = spool2.tile([P, NC], F32, tag="f2tmp")
                        nc.vector.tensor_scalar_add(tmp, ps, b2_s[:, m:m + 1])
                        nc.vector.tensor_tensor(
                            x[:, m, cs], tmp, x[:, m, cs], OP.add)

            # ---- final LN + LM head ----
            layernorm()
            for m in range(MV):
                we_m = w6(wemb.ap(), m)
                for c in range(NCH):
                    cs = slice(c * NC, (c + 1) * NC)
                    ps = psp.tile([P, NC], F32, tag="mm")
                    for k in range(KE):
                        nc.tensor.matmul(
                            ps, we_m[:, k, :], xhat[:, k, cs],
                            start=(k == 0), stop=(k == KE - 1))
                    ot = spool2.tile([P, NC], F32, tag="outsb")
                    nc.vector.tensor_copy(out=ot, in_=ps)
                    nc.sync.dma_start(out.ap()[m * P:(m + 1) * P, cs], ot)

    nc.compile()
    return nc


def _prep(inputs):
    """Host-side: fold LN scales into weights, build per-core input maps."""
    ids = np.asarray(inputs["input_ids"]).astype(np.int64)
    tok = np.asarray(inputs["tok_emb"], np.float32)
    pos = np.asarray(inputs["pos_emb"], np.float32)
    qkv = np.asarray(inputs["qkv_w"], np.float32)
    ow = np.asarray(inputs["out_w"], np.float32)
    f1 = np.asarray(inputs["fc1_w"], np.float32)
    b1 = np.asarray(inputs["fc1_b"], np.float32)
    f2 = np.asarray(inputs["fc2_w"], np.float32)
    b2 = np.asarray(inputs["fc2_b"], np.float32)
    s1 = np.asarray(inputs["ln1_scale"], np.float32)
    bb1 = np.asarray(inputs["ln1_bias"], np.float32)
    s2 = np.asarray(inputs["ln2_scale"], np.float32)
    bb2 = np.asarray(inputs["ln2_bias"], np.float32)
    sf = np.asarray(inputs["lnf_scale"], np.float32)
    bf_ = np.asarray(inputs["lnf_bias"], np.float32)
    # LN biases must be zero for the fold used here (true for this model).
    assert abs(bb1).max() == 0 and abs(bb2).max() == 0 and abs(bf_).max() == 0

    x0 = tok[ids] + pos[None, :, :]                      # (B, T, E)
    x0t = np.ascontiguousarray(x0.transpose(0, 2, 1))    # (B, E, T)

    scale = HD ** -0.5
    wqk_h = np.empty((L, E, 2 * E), BF)
    wv_h = np.empty((L, E, E), BF)
    wo_h = np.empty((L, E, E), BF)
    w1_h = np.empty((L, E, FF), BF)
    w2_h = np.empty((L, FF, E), BF)
    b1_h = np.zeros((L, P, KF), np.float32)
    b2_h = np.zeros((L, P, KE), np.float32)
    for l in range(L):
        wq = (qkv[l, :E] * s1[l][None, :]).T * scale
        wk = (qkv[l, E:2 * E] * s1[l][None, :]).T
        wv_ = (qkv[l, 2 * E:] * s1[l][None, :]).T
        wqk_h[l] = np.concatenate([wq, wk], axis=1).astype(BF)
        wv_h[l] = wv_.astype(BF)
        wo_h[l] = ow[l].T.astype(BF)
        w1_h[l] = (f1[l] * s2[l][None, :]).T.astype(BF)
        w2_h[l] = f2[l].T.astype(BF)
        b1_h[l] = b1[l].reshape(KF, P).T
        b2_h[l] = b2[l].reshape(KE, P).T

    tokp = np.zeros((4 * VP, E), np.float32)
    tokp[:V] = tok * sf[None, :]
    embt = [np.ascontiguousarray(tokp[j * VP:(j + 1) * VP].T).astype(BF)
            for j in range(4)]

    # 4 diagonal-crossing masks: d = 0,128,256,384 partition offset
    m = np.zeros((4, P, NC), np.float32)
    for i in range(4):
        gk = i * P + np.arange(P)[:, None]
        m[i] = (gk <= np.arange(NC)[None, :])
    mask_h = m.astype(BF)

    in_maps = []
    for c in range(8):
        g, j = c // 4, c % 4
        in_maps.append({
            "x0t": np.ascontiguousarray(x0t[g]),
            "wqk": wqk_h, "wv": wv_h, "wout": wo_h,
            "wfc1": w1_h, "bfc1": b1_h, "wfc2": w2_h, "bfc2": b2_h,
            "wemb": embt[j], "mask": mask_h,
        })
    return in_maps


def kernel(**inputs) -> np.ndarray:
    if "nc" not in _CACHE:
        _CACHE["nc"] = _build()
    nc = _CACHE["nc"]
    in_maps = _prep(inputs)
    res = run_bass_kernel_spmd(nc, in_maps, list(range(8)),
                               **_CACHE.get("run_kwargs", {}))
    _CACHE["last"] = res
    logits = np.empty((B, T, V), np.float32)
    for c in range(8):
        g, j = c // 4, c % 4
        lo = j * VP
        hi = min(V, lo + VP)
        logits[g, :, lo:hi] = res.results[c]["out"][:hi - lo].T
    return logits



# revision 11
# speedup vs baseline: 2.0803x; 2.0803x over previous
"""Distributed Trainium2 Bass kernel for a 4-layer GPT-style transformer.

Sharding: 8 cores = 2 batch groups x 4-way sequence parallel.
  - core c: batch element g = c//4, token slice r = c%4 (tokens
    256r..256r+255), vocab shard r for the tied LM head.
  - Per layer each core projects Q/K/V for its 256 tokens, AllGathers
    K^T and V across its group of 4, runs causal attention for its
    queries against all keys, then out-proj + FFN token-parallel.
  - Final hidden states are AllGathered so every core computes its
    vocab shard of the LM head over all 1024 tokens.

On-chip layout: residual stream transposed (features on partitions,
tokens on free).  Scores computed keys-on-partitions (k @ q^T); the
AV matmul is flipped to queries-on-partitions with a ones-column
appended to V so the softmax denominator falls out of the same
accumulation for free.  Causality = 0/1 mask multiply after exp (mask
is per-core input data, keeping the SPMD graph uniform).  Matmuls
bf16, residual stream fp32.  Weights streamed from DRAM per output
tile in stationary-tile-major layout (contiguous DMA).
"""

import numpy as np
import ml_dtypes

import concourse.bass as bass
import concourse.mybir as mybir
import concourse.tile as tile
from concourse import bacc
from concourse.bass_utils import run_bass_kernel_spmd
from concourse.masks import make_identity

V, E, NH, HD, L, T, B, FF = 50257, 768, 12, 64, 4, 1024, 2, 3072
EPS = 1e-5
P = 128
KE = E // P            # 6 feature subtiles
KF = FF // P           # 24
TL = 256               # local tokens per core
NT = T // P            # 8 key tiles
VP = 12672             # vocab shard per core (99 * 128)
MV = VP // P           # 99
RG = [[0, 1, 2, 3], [4, 5, 6, 7]]
BF16 = mybir.dt.bfloat16
F32 = mybir.dt.float32
AF = mybir.ActivationFunctionType
OP = mybir.AluOpType
BF = ml_dtypes.bfloat16

_CACHE = {}


def _build():
    nc = bacc.Bacc("TRN2", target_bir_lowering=False, debug=False,
                   num_devices=8)

    x0t = nc.declare_dram_parameter("x0t", [E, TL], F32, isOutput=False)
    wqk = nc.declare_dram_parameter("wqk", [L, 2 * KE, P, KE * P], BF16,
                                    isOutput=False)
    wv = nc.declare_dram_parameter("wv", [L, P, KE * E], BF16, isOutput=False)
    wout = nc.declare_dram_parameter("wout", [L, KE, P, KE * P], BF16,
                                     isOutput=False)
    wfc1 = nc.declare_dram_parameter("wfc1", [L, KF, P, KE * P], BF16,
                                     isOutput=False)
    bfc1 = nc.declare_dram_parameter("bfc1", [L, P, KF], F32, isOutput=False)
    wfc2 = nc.declare_dram_parameter("wfc2", [L, KE, P, KF * P], BF16,
                                     isOutput=False)
    bfc2 = nc.declare_dram_parameter("bfc2", [L, P, KE], F32, isOutput=False)
    wemb = nc.declare_dram_parameter("wemb", [MV, P, KE * P], BF16,
                                     isOutput=False)
    maskp = nc.declare_dram_parameter("mask", [NT, P, TL], BF16,
                                      isOutput=False)
    out = nc.declare_dram_parameter("out", [MV, P, T], F32, isOutput=True)

    with tile.TileContext(nc) as tc:
        with (
            tc.tile_pool(name="resident", bufs=1) as res,
            tc.tile_pool(name="wts", bufs=2) as wpool,
            tc.tile_pool(name="wstream", bufs=3) as wst,
            tc.tile_pool(name="acts", bufs=2) as apool,
            tc.tile_pool(name="small", bufs=3) as spool,
            tc.tile_pool(name="dram", bufs=2, space="DRAM") as dpool,
            tc.tile_pool(name="ps", bufs=1, space="PSUM") as psp,
        ):
            # --- resident tiles ---
            x = res.tile([P, KE, TL], F32)         # residual stream (xT)
            xhat = res.tile([P, KE, TL], BF16)     # normalized, bf16
            mask = res.tile([P, NT, TL], BF16)     # causal masks (per-core)
            ones_c = res.tile([P, 1], BF16)
            ones_r = res.tile([1, P], F32)
            eps_c = res.tile([1, 1], F32)
            ident = res.tile([P, P], BF16)
            nr_b = res.tile([P, 2, TL], F32)   # -mean / rstd broadcast
            q_s = res.tile([P, KE, TL], BF16)      # Q^T local
            kq_l = res.tile([P, KE, TL], BF16)     # K^T local (pre-gather)
            v_l = res.tile([P, 2, E], BF16)        # V local (tok, E)
            kg = res.tile([P, KE, 4, TL], BF16)    # K^T gathered
            vg = res.tile([P, NT, NH, HD + 1], BF16)  # V gathered + ones col
            o_q = res.tile([P, 2, E], BF16)        # attn out, queries on part
            o_t = res.tile([P, KE, TL], BF16)      # attn out, transposed
            h1 = res.tile([P, KF, TL], BF16)       # FFN hidden
            xf = res.tile([P, KE, 4, TL], BF16)    # final hidden, gathered

            nc.any.memset(ones_c[:], 1.0)
            nc.any.memset(ones_r[:], 1.0)
            nc.any.memset(eps_c[:], EPS)
            nc.any.memset(vg[:, :, :, HD:HD + 1], 1.0)
            make_identity(nc, ident[:])
            nc.sync.dma_start(mask[:], maskp.ap().rearrange("n p t -> p n t"))
            nc.sync.dma_start(x[:], x0t.ap().rearrange("(ko p) t -> p ko t",
                                                       p=P))

            def layernorm():
                """x (f32) -> xhat (bf16), pure normalize (scales folded).

                Sum and sum-of-squares go to SEPARATE PSUM banks:
                interleaved multi-instruction accumulation groups sharing
                one bank corrupt each other on hardware."""
                ps_s = psp.tile([P, 2, TL], F32, tag="sc", bufs=3)
                ps_q = psp.tile([P, 2, TL], F32, tag="sc", bufs=3)
                for k in range(KE):
                    xbt = spool.tile([P, TL], BF16, tag="xbt")
                    nc.vector.tensor_copy(out=xbt[:], in_=x[:, k, :])
                    nc.tensor.matmul(ps_s[:1, 0, :], ones_c[:], xbt[:],
                                     start=(k == 0), stop=(k == KE - 1))
                    xsq = spool.tile([P, TL], BF16, tag="xsq")
                    nc.vector.tensor_tensor(xsq[:], xbt[:], xbt[:], OP.mult)
                    nc.tensor.matmul(ps_q[:1, 0, :], ones_c[:], xsq[:],
                                     start=(k == 0), stop=(k == KE - 1))
                nm = spool.tile([1, TL], F32, tag="nm")
                t_m = spool.tile([1, TL], F32, tag="t_m")
                t_v = spool.tile([1, TL], F32, tag="t_v")
                nc.vector.tensor_scalar_mul(nm, ps_s[:1, 0, :], -1.0 / E)
                nc.vector.tensor_scalar_mul(t_m, ps_s[:1, 0, :], 1.0 / E)
                nc.vector.tensor_scalar_mul(t_v, ps_q[:1, 0, :], 1.0 / E)
                nc.vector.tensor_tensor(t_m, t_m, t_m, OP.mult)
                nc.vector.tensor_tensor(t_v, t_v, t_m, OP.subtract)
                nc.scalar.activation(t_v, t_v, AF.Sqrt, bias=eps_c[:])
                nc.vector.reciprocal(t_m, t_v)
                ps_b = psp.tile([P, 512], F32, tag="mm", bufs=2)
                nc.tensor.matmul(ps_b[:, :TL], ones_r[:], nm,
                                 start=True, stop=True)
                nc.tensor.matmul(ps_b[:, TL:], ones_r[:], t_m,
                                 start=True, stop=True)
                nc.vector.tensor_copy(out=nr_b[:, 0, :], in_=ps_b[:, :TL])
                nc.vector.tensor_copy(out=nr_b[:, 1, :], in_=ps_b[:, TL:])
                for k in range(KE):
                    tmp = spool.tile([P, TL], F32, tag="lnt")
                    nc.vector.tensor_tensor(tmp, x[:, k, :], nr_b[:, 0, :],
                                            OP.add)
                    nc.vector.tensor_tensor(xhat[:, k, :], tmp, nr_b[:, 1, :],
                                            OP.mult)

            def proj6(dram_l, dst, m0):
                """6 stationary tiles of dram_l -> dst [P, KE, TL] bf16."""
                for j in range(KE):
                    wt = wst.tile([P, KE * P], BF16, tag="w6")
                    nc.sync.dma_start(wt[:], dram_l[m0 + j])
                    ps = psp.tile([P, 512], F32, tag="mm", bufs=2)
                    for k in range(KE):
                        nc.tensor.matmul(
                            ps[:, :TL], wt[:, k * P:(k + 1) * P],
                            xhat[:, k, :], start=(k == 0), stop=(k == KE - 1))
                    nc.vector.tensor_copy(out=dst[:, j, :], in_=ps[:, :TL])

            for l in range(L):
                wv_t = wpool.tile([P, KE * E], BF16, tag="wv")
                b1_t = wpool.tile([P, KF], F32, tag="b1")
                b2_t = wpool.tile([P, KE], F32, tag="b2")
                nc.sync.dma_start(wv_t[:], wv.ap()[l])
                nc.sync.dma_start(b1_t[:], bfc1.ap()[l])
                nc.sync.dma_start(b2_t[:], bfc2.ap()[l])

                layernorm()

                # ---- K projection + AllGather ----
                proj6(wqk.ap()[l], kq_l, KE)
                kb_i = dpool.tile([E, TL], BF16, tag="kbi")
                kb_o = dpool.tile([4 * E, TL], BF16, tag="kbo")
                nc.gpsimd.dma_start(
                    kb_i.rearrange("(ko p) t -> p ko t", p=P), kq_l[:])
                nc.gpsimd.collective_compute(
                    "AllGather", OP.bypass, replica_groups=RG,
                    ins=[kb_i.opt()], outs=[kb_o.opt()])

                # ---- V projection + AllGather ----
                for tt in range(2):
                    for (f0, fn) in ((0, 512), (512, E - 512)):
                        ps = psp.tile([P, 512], F32, tag="mm", bufs=2)
                        for k in range(KE):
                            nc.tensor.matmul(
                                ps[:, :fn], xhat[:, k, tt * P:(tt + 1) * P],
                                wv_t[:, k * E + f0:k * E + f0 + fn],
                                start=(k == 0), stop=(k == KE - 1))
                        nc.vector.tensor_copy(
                            out=v_l[:, tt, f0:f0 + fn], in_=ps[:, :fn])
                vb_i = dpool.tile([TL, E], BF16, tag="vbi")
                vb_o = dpool.tile([T, E], BF16, tag="vbo")
                nc.gpsimd.dma_start(
                    vb_i.rearrange("(tt p) f -> p tt f", p=P), v_l[:])
                nc.gpsimd.collective_compute(
                    "AllGather", OP.bypass, replica_groups=RG,
                    ins=[vb_i.opt()], outs=[vb_o.opt()])

                # ---- Q projection (overlaps the gathers) ----
                proj6(wqk.ap()[l], q_s, 0)

                # ---- land gathered K^T and V ----
                for b in range(4):
                    nc.gpsimd.dma_start(
                        kg[:, :, b, :],
                        kb_o[b * E:(b + 1) * E, :].rearrange(
                            "(ko p) t -> p ko t", p=P))
                for kt in range(NT):
                    nc.gpsimd.dma_start(
                        vg[:, kt, :, 0:HD],
                        vb_o[kt * P:(kt + 1) * P, :].rearrange(
                            "p (h d) -> p h d", h=NH))

                # ---- attention per head ----
                for h in range(NH):
                    mt, mo = divmod(h * HD, P)
                    pt = apool.tile([P, NT, TL], BF16, tag="pt")
                    for kk in range(NT // 2):
                        ps_sc = psp.tile([P, 2, TL], F32, tag="sc", bufs=3)
                        for j in range(2):
                            kt = 2 * kk + j
                            ko_b, ko_o = kt // 2, (kt % 2) * P
                            nc.tensor.matmul(
                                ps_sc[:, j, :],
                                kg[mo:mo + HD, mt, ko_b, ko_o:ko_o + P],
                                q_s[mo:mo + HD, mt, :],
                                start=True, stop=True)
                        nc.scalar.activation(
                            pt[:, 2 * kk:2 * kk + 2, :], ps_sc[:], AF.Exp)
                        nc.vector.tensor_tensor(
                            pt[:, 2 * kk:2 * kk + 2, :],
                            pt[:, 2 * kk:2 * kk + 2, :],
                            mask[:, 2 * kk:2 * kk + 2, :], OP.mult)
                    ps_av = psp.tile([P, 2, P], F32, tag="sm", bufs=2)
                    for qt in range(2):
                        for kt in range(NT):
                            nc.tensor.matmul(
                                ps_av[:, qt, :HD + 1],
                                pt[:, kt, qt * P:(qt + 1) * P],
                                vg[:, kt, h, :],
                                start=(kt == 0), stop=(kt == NT - 1))
                    for qt in range(2):
                        rd = spool.tile([P, 1], F32, tag="rd")
                        nc.vector.reciprocal(rd, ps_av[:, qt, HD:HD + 1])
                        nc.vector.tensor_scalar_mul(
                            o_q[:, qt, h * HD:(h + 1) * HD],
                            ps_av[:, qt, :HD], rd)

                # ---- transpose attn output to feature-major ----
                for qt in range(2):
                    ps_t = psp.tile([P, KE, P], BF16, tag="tr", bufs=1)
                    for ko in range(KE):
                        nc.tensor.transpose(
                            ps_t[:, ko, :], o_q[:, qt, ko * P:(ko + 1) * P],
                            ident[:])
                    nc.vector.tensor_copy(
                        out=o_t[:, :, qt * P:(qt + 1) * P], in_=ps_t[:])

                # ---- output projection + residual ----
                for m in range(KE):
                    wt = wst.tile([P, KE * P], BF16, tag="w6")
                    nc.sync.dma_start(wt[:], wout.ap()[l, m])
                    ps = psp.tile([P, 512], F32, tag="mm", bufs=2)
                    for k in range(KE):
                        nc.tensor.matmul(
                            ps[:, :TL], wt[:, k * P:(k + 1) * P], o_t[:, k, :],
                            start=(k == 0), stop=(k == KE - 1))
                    nc.vector.tensor_tensor(
                        x[:, m, :], ps[:, :TL], x[:, m, :], OP.add)

                layernorm()

                # ---- FFN ----
                for m in range(KF):
                    wt = wst.tile([P, KE * P], BF16, tag="w6")
                    nc.sync.dma_start(wt[:], wfc1.ap()[l, m])
                    ps = psp.tile([P, 512], F32, tag="mm", bufs=2)
                    for k in range(KE):
                        nc.tensor.matmul(
                            ps[:, :TL], wt[:, k * P:(k + 1) * P],
                            xhat[:, k, :], start=(k == 0), stop=(k == KE - 1))
                    nc.scalar.activation(
                        h1[:, m, :], ps[:, :TL], AF.Gelu,
                        bias=b1_t[:, m:m + 1])
                for m in range(KE):
                    wt24 = wst.tile([P, KF * P], BF16, tag="w24")
                    nc.sync.dma_start(wt24[:], wfc2.ap()[l, m])
                    ps = psp.tile([P, 512], F32, tag="mm", bufs=2)
                    for k in range(KF):
                        nc.tensor.matmul(
                            ps[:, :TL], wt24[:, k * P:(k + 1) * P],
                            h1[:, k, :], start=(k == 0), stop=(k == KF - 1))
                    tmp = spool.tile([P, TL], F32, tag="f2t")
                    nc.vector.tensor_scalar_add(tmp, ps[:, :TL],
                                                b2_t[:, m:m + 1])
                    nc.vector.tensor_tensor(
                        x[:, m, :], tmp, x[:, m, :], OP.add)

            # ---- final LN + gather + LM head ----
            layernorm()
            xf_i = dpool.tile([E, TL], BF16, tag="kbi")
            xf_o = dpool.tile([4 * E, TL], BF16, tag="kbo")
            nc.gpsimd.dma_start(
                xf_i.rearrange("(ko p) t -> p ko t", p=P), xhat[:])
            nc.gpsimd.collective_compute(
                "AllGather", OP.bypass, replica_groups=RG,
                ins=[xf_i.opt()], outs=[xf_o.opt()])
            for b in range(4):
                nc.gpsimd.dma_start(
                    xf[:, :, b, :],
                    xf_o[b * E:(b + 1) * E, :].rearrange(
                        "(ko p) t -> p ko t", p=P))
            for m in range(MV):
                we = wst.tile([P, KE * P], BF16, tag="we")
                nc.sync.dma_start(we[:], wemb.ap()[m])
                for c in range(2):
                    cs = slice(c * 512, (c + 1) * 512)
                    ps = psp.tile([P, 512], F32, tag="mm", bufs=2)
                    for k in range(KE):
                        nc.tensor.matmul(
                            ps, we[:, k * P:(k + 1) * P],
                            xf[:, k, 2 * c:2 * c + 2, :],
                            start=(k == 0), stop=(k == KE - 1))
                    ot = apool.tile([P, 512], F32, tag="ot")
                    nc.scalar.copy(ot, ps)
                    nc.sync.dma_start(out.ap()[m][:, cs], ot)

    nc.compile()
    return nc


def _prep(inputs):
    """Host-side: fold LN scales into weights, build per-core input maps."""
    ids = np.asarray(inputs["input_ids"]).astype(np.int64)
    tok = np.asarray(inputs["tok_emb"], np.float32)
    pos = np.asarray(inputs["pos_emb"], np.float32)
    qkv = np.asarray(inputs["qkv_w"], np.float32)
    ow = np.asarray(inputs["out_w"], np.float32)
    f1 = np.asarray(inputs["fc1_w"], np.float32)
    b1 = np.asarray(inputs["fc1_b"], np.float32)
    f2 = np.asarray(inputs["fc2_w"], np.float32)
    b2 = np.asarray(inputs["fc2_b"], np.float32)
    s1 = np.asarray(inputs["ln1_scale"], np.float32)
    bb1 = np.asarray(inputs["ln1_bias"], np.float32)
    s2 = np.asarray(inputs["ln2_scale"], np.float32)
    bb2 = np.asarray(inputs["ln2_bias"], np.float32)
    sf = np.asarray(inputs["lnf_scale"], np.float32)
    bf_ = np.asarray(inputs["lnf_bias"], np.float32)
    # LN biases must be zero for the fold used here (true for this model).
    assert abs(bb1).max() == 0 and abs(bb2).max() == 0 and abs(bf_).max() == 0

    x0 = tok[ids] + pos[None, :, :]                      # (B, T, E)
    x0t = np.ascontiguousarray(x0.transpose(0, 2, 1))    # (B, E, T)

    scale = HD ** -0.5
    wqk_h = np.empty((L, 2 * KE, P, KE * P), BF)
    wv_h = np.empty((L, P, KE * E), BF)
    wo_h = np.empty((L, KE, P, KE * P), BF)
    w1_h = np.empty((L, KF, P, KE * P), BF)
    w2_h = np.empty((L, KE, P, KF * P), BF)
    b1_h = np.zeros((L, P, KF), np.float32)
    b2_h = np.zeros((L, P, KE), np.float32)

    def tiles(w, nm):
        # w: [E_in, n*P] (contract rows, out cols) -> [n, P, (E_in/P)*P]
        ki = w.shape[0] // P
        return np.ascontiguousarray(
            w.reshape(ki, P, nm, P).transpose(2, 1, 0, 3).reshape(
                nm, P, ki * P))

    for l in range(L):
        wq = (qkv[l, :E] * s1[l][None, :]).T * scale
        wk = (qkv[l, E:2 * E] * s1[l][None, :]).T
        wqk_h[l] = tiles(np.concatenate([wq, wk], axis=1), 2 * KE).astype(BF)
        wv_ = (qkv[l, 2 * E:] * s1[l][None, :]).T          # [E, E]
        wv_h[l] = wv_.reshape(KE, P, E).transpose(1, 0, 2).reshape(
            P, KE * E).astype(BF)
        wo_h[l] = tiles(ow[l].T, KE).astype(BF)
        w1_h[l] = tiles((f1[l] * s2[l][None, :]).T, KF).astype(BF)
        w2_h[l] = tiles(f2[l].T, KE).astype(BF)
        b1_h[l] = b1[l].reshape(KF, P).T
        b2_h[l] = b2[l].reshape(KE, P).T

    tokp = np.zeros((4 * VP, E), np.float32)
    tokp[:V] = tok * sf[None, :]
    emb_h = [tiles(np.ascontiguousarray(tokp[j * VP:(j + 1) * VP].T), MV)
             .astype(BF) for j in range(4)]

    # causal masks per token-slice r: key kt*128+kk visible to query
    # r*256+qq iff key <= query
    mask_h = []
    for r in range(4):
        gk = np.arange(NT * P)[:, None]
        gq = r * TL + np.arange(TL)[None, :]
        mask_h.append((gk <= gq).reshape(NT, P, TL).astype(BF))

    in_maps = []
    for c in range(8):
        g, r = c // 4, c % 4
        in_maps.append({
            "x0t": np.ascontiguousarray(x0t[g][:, r * TL:(r + 1) * TL]),
            "wqk": wqk_h, "wv": wv_h, "wout": wo_h,
            "wfc1": w1_h, "bfc1": b1_h, "wfc2": w2_h, "bfc2": b2_h,
            "wemb": emb_h[r], "mask": mask_h[r],
        })
    return in_maps


def kernel(**inputs) -> np.ndarray:
    if "nc" not in _CACHE:
        _CACHE["nc"] = _build()
    nc = _CACHE["nc"]
    in_maps = _prep(inputs)
    res = run_bass_kernel_spmd(nc, in_maps, list(range(8)),
                               **_CACHE.get("run_kwargs", {}))
    _CACHE["last"] = res
    logits = np.empty((B, T, V), np.float32)
    for c in range(8):
        g, r = c // 4, c % 4
        lo = r * VP
        hi = min(V, lo + VP)
        shard = res.results[c]["out"].reshape(VP, T)
        logits[g, :, lo:hi] = shard[:hi - lo].T
    return logits
